# revision 1
# baseline (speedup 1.0000x reference)
"""Trainium2 Bass kernel for nn_MatrixFactorization (segment_reduce).

Decomposition (8 cores, SPMD, no collectives):
  - Dedup users of the batch -> unique users, sharded 8 ways (upc per core).
  - Host shards train_label[uniq].T per core in partition-major layout
    (contraction dim = items lands on SBUF partitions), zero-padded to
    157*128 rows.
  - Device streams the f32 label shard with SWDGE cast-DMA to bf16 (labels
    are exactly 0/1, so the cast is lossless) and accumulates
        P.T[66, upc] += T_aug_chunk.T @ L.T_chunk    (157 chunks of K=128)
    where T_aug = [item_table (row 20000 zeroed) | ones | 0] in bf16; column
    64 of P accumulates num_rel. PE transposes P.T back per 128-user block,
    DVE computes P[:, :64] * recip(P[:, 64]) -> uni_center rows.
  - Cluster centers: per chunk, DVE iota/is_equal builds onehot[128k, 256c];
    PE accumulates centers.T[66, 256] using the same T_aug chunks (the last
    chunk uses a variant with the real row 20000). counts ride the ones
    column; finalize = transpose, max(count,1), reciprocal, scale.
    Replicated on every core (hidden under the DMA-bound main loop).
  - user/pos/neg embeddings: GPSIMD indirect row gathers (256B rows),
    interleaved with the streaming loop.
  - pos/neg centers: exact fp32 onehot matmuls against the finalized
    centers (out as [64, nbpc]; host transposes back). Only one nonzero per
    onehot column, so this is an exact gather.
"""

import numpy as np
import ml_dtypes

import concourse.bass as bass
import concourse.mybir as mybir
import concourse.tile as tile

NUM_USERS = 10000
NUM_ITEMS = 20000
DIM = 64
CLUSTER = 256
BATCH = 8192
NCORES = 8

KCHUNKS = 157            # ceil(20001 / 128)
KPAD = KCHUNKS * 128     # 20096
MAUG = 66                # 64 dims + ones col + pad col
GROUP_SIZES = [2, 4, 8] + [16] * 8 + [10, 4, 1]  # staggered spin-up/down
assert sum(GROUP_SIZES) == KCHUNKS


def split_multiwaits(nc):
    """nix-walrus accepts at most ONE sync-wait per instruction; Tile attaches
    many. Hoist all but the last wait onto single-wait NoOps inserted just
    before the instruction, on the same engine."""
    n_split = 0
    for f in nc.m.functions:
        for bb in f.blocks:
            il = list(bb.instructions)
            new = []
            changed = False
            for ins in il:
                si = ins.sync_info
                if si is not None and si.on_wait is not None and len(si.on_wait) > 1:
                    waits = list(si.on_wait)
                    for k, w in enumerate(waits[:-1]):
                        nop = mybir.InstNoOp(
                            name=f"{ins.name}-wsplit{k}", ins=[], outs=[]
                        )
                        nop.engine = ins.engine
                        nop.sync_info = mybir.SyncInfo(on_wait=[w], on_update=[])
                        new.append(nop)
                    ins.sync_info = mybir.SyncInfo(
                        on_wait=waits[-1:], on_update=list(si.on_update or [])
                    )
                    changed = True
                    n_split += 1
                new.append(ins)
            if changed:
                bb.instructions = new
    return n_split


def build_bass(upc: int, nbpc: int):
    """upc: unique users per core; nbpc: batch entries per core."""
    f32 = mybir.dt.float32
    bf16 = mybir.dt.bfloat16
    f16 = mybir.dt.float16
    i32 = mybir.dt.int32
    EQ = mybir.AluOpType.is_equal
    MUL = mybir.AluOpType.mult

    assert nbpc % 128 == 0
    jg = nbpc // 128

    nc = bass.Bass(trn_type="TRN2")

    # ---- I/O ----
    # lt is partition-major: lt[p, c, u] = label.T[c*128 + p, u]
    LT = nc.dram_tensor("lt", [128, KCHUNKS, upc], f32, kind="ExternalInput")
    T_pm = nc.dram_tensor("t_pm", [128, KCHUNKS * MAUG], bf16, kind="ExternalInput")
    T_cl = nc.dram_tensor("t_cl", [128, MAUG], bf16, kind="ExternalInput")
    EYE = nc.dram_tensor("eye66", [MAUG, MAUG], f32, kind="ExternalInput")
    IOTA = nc.dram_tensor("iota256", [128, CLUSTER], f32, kind="ExternalInput")
    CID = nc.dram_tensor("cid_pm", [128, KCHUNKS], f32, kind="ExternalInput")
    PCOL = nc.dram_tensor("pcol", [128, 2], f32, kind="ExternalInput")
    CPR = nc.dram_tensor("cpr", [128, nbpc], bf16, kind="ExternalInput")
    CNR = nc.dram_tensor("cnr", [128, nbpc], bf16, kind="ExternalInput")
    UT = nc.dram_tensor("user_table", [NUM_USERS, DIM], f32, kind="ExternalInput")
    IT = nc.dram_tensor("item_table", [NUM_ITEMS + 1, DIM], f32, kind="ExternalInput")
    IDX = {}
    for nm in ("uidx", "pidx", "nidx"):
        IDX[nm] = nc.dram_tensor(nm, [128, jg], i32, kind="ExternalInput")

    UNI = nc.dram_tensor("uni_part", [upc, DIM], f32, kind="ExternalOutput")
    EMB = {}
    for nm in ("ue_out", "pe_out", "ne_out"):
        EMB[nm] = nc.dram_tensor(nm, [nbpc, DIM], f32, kind="ExternalOutput")
    PCT = nc.dram_tensor("pct_out", [DIM, nbpc], f32, kind="ExternalOutput")
    NCT = nc.dram_tensor("nct_out", [DIM, nbpc], f32, kind="ExternalOutput")

    blocks = []
    o = 0
    while o < upc:
        blocks.append((o, min(128, upc - o)))
        o += 128
    nA = min(512, upc)
    gmax = max(GROUP_SIZES)

    with tile.TileContext(nc) as tc:
        with (
            tc.tile_pool(name="const", bufs=1) as cpool,
            tc.tile_pool(name="ltp", bufs=2) as ltpool,
            tc.tile_pool(name="ohp", bufs=2) as ohpool,
            tc.tile_pool(name="acc", bufs=1, space="PSUM") as accpool,
            tc.tile_pool(name="tp", bufs=2, space="PSUM") as tppool,
            tc.tile_pool(name="outp", bufs=3) as outpool,
        ):
            # ---- constants into SBUF ----
            # Two HWDGE rings: T_aug pieces on the sync ring (piecewise, so
            # early chunks' weights arrive with the early lt groups); every
            # small constant on the scalar ring so nothing queues behind the
            # 2.6MB T_aug stream.
            t_sb = cpool.tile([128, KCHUNKS, MAUG], bf16)
            t_view = T_pm[:].rearrange("p (c m) -> p c m", m=MAUG)
            tb = 0
            for piece in (GROUP_SIZES[0], GROUP_SIZES[1], GROUP_SIZES[2],
                          16, KCHUNKS):
                te = min(tb + piece, KCHUNKS)
                nc.sync.dma_start(t_sb[:, tb:te, :], t_view[:, tb:te, :])
                tb = te
                if tb == KCHUNKS:
                    break
            iota_sb = cpool.tile([128, CLUSTER], f32)
            nc.scalar.dma_start(iota_sb[:], IOTA[:])
            cid_sb = cpool.tile([128, KCHUNKS], f32)
            nc.scalar.dma_start(cid_sb[:], CID[:])
            tcl_sb = cpool.tile([128, MAUG], bf16)
            nc.scalar.dma_start(tcl_sb[:], T_cl[:])
            pcol_sb = cpool.tile([128, 2], f32)
            nc.scalar.dma_start(pcol_sb[:], PCOL[:])
            idx_sb = {}
            g_sb = {}
            for nm, h in IDX.items():
                s = cpool.tile([128, jg], i32, name=f"idx_{nm}")
                nc.scalar.dma_start(s[:], h[:])
                idx_sb[nm] = s
                g_sb[nm] = cpool.tile([128, jg, DIM], f32, name=f"g_{nm}")
            eye_sb = cpool.tile([MAUG, MAUG], f32)
            nc.scalar.dma_start(eye_sb[:], EYE[:])
            cpr_sb = cpool.tile([128, nbpc], bf16)
            nc.scalar.dma_start(cpr_sb[:], CPR[:])
            cnr_sb = cpool.tile([128, nbpc], bf16)
            nc.scalar.dma_start(cnr_sb[:], CNR[:])

            # one [128]-row slice of an embedding gather
            gsrc = {"uidx": UT[:], "pidx": IT[:], "nidx": IT[:]}

            def gather_slice(nm, j):
                nc.gpsimd.indirect_dma_start(
                    out=g_sb[nm][:, j, :],
                    out_offset=None,
                    in_=gsrc[nm],
                    in_offset=bass.IndirectOffsetOnAxis(
                        ap=idx_sb[nm][:, j : j + 1], axis=0
                    ),
                )

            gather_slices = [(nm, j) for nm in ("uidx", "pidx", "nidx")
                             for j in range(jg)]

            # ---- psum accumulators ----
            accA = accpool.tile([MAUG, nA], f32)
            accB = (
                accpool.tile([MAUG, upc - nA], f32, name="accB")
                if upc > nA
                else None
            )
            accC = accpool.tile([MAUG, CLUSTER], f32)

            lt_view = LT[:]

            # ---- main streaming loop ----
            c0 = 0
            n_groups = len(GROUP_SIZES)
            for g, gs in enumerate(GROUP_SIZES):
                lt = ltpool.tile([128, gmax, upc], bf16, name="lt_tile")
                nc.gpsimd.dma_start(lt[:, 0:gs, :], lt_view[:, c0 : c0 + gs, :])
                # spread the embedding-row gathers through the stream so
                # their descriptor generation hides under the big loads
                if g >= 1:
                    k0 = (g - 1) * len(gather_slices) // (n_groups - 1)
                    k1 = g * len(gather_slices) // (n_groups - 1)
                    for nm, j in gather_slices[k0:k1]:
                        gather_slice(nm, j)
                for j in range(gs):
                    c = c0 + j
                    st = c == 0
                    sp = c == KCHUNKS - 1
                    lhs = t_sb[:, c, :]
                    nc.tensor.matmul(
                        accA[:], lhs, lt[:, j, 0:nA], start=st, stop=sp
                    )
                    if accB is not None:
                        nc.tensor.matmul(
                            accB[:], lhs, lt[:, j, nA:upc], start=st, stop=sp
                        )
                    oh = ohpool.tile([128, CLUSTER], bf16, name="oh")
                    nc.vector.tensor_scalar(
                        oh[:], iota_sb[:], cid_sb[:, c : c + 1], None, EQ
                    )
                    nc.tensor.matmul(
                        accC[:],
                        tcl_sb[:] if sp else lhs,
                        oh[:],
                        start=st,
                        stop=sp,
                    )
                c0 += gs

            # ---- embedding gather writeback ----
            for nm, out in (("uidx", EMB["ue_out"]), ("pidx", EMB["pe_out"]),
                            ("nidx", EMB["ne_out"])):
                nc.scalar.dma_start(
                    out[:].rearrange("(j p) d -> p j d", p=128), g_sb[nm][:]
                )

            # ---- finalize centers (emitted first: longest tail chain) ----
            c_sb = outpool.tile([MAUG, CLUSTER], f32, bufs=1)
            nc.vector.tensor_copy(c_sb[:], accC[:])
            nc.vector.tensor_scalar(
                c_sb[64:65, :], c_sb[64:65, :], 1.0, None, mybir.AluOpType.max
            )
            ce = []
            for h in range(2):
                ctp = tppool.tile([128, MAUG], f32, name=f"ctp{h}", tag="tps")
                nc.tensor.matmul(
                    ctp[:], c_sb[:, h * 128 : (h + 1) * 128], eye_sb[:],
                    is_transpose=True,
                )
                rc = outpool.tile([128, 1], f32, name=f"rc{h}")
                nc.vector.reciprocal(rc[:], ctp[:, 64:65])
                ce_sb = outpool.tile([128, DIM], f16, name=f"ce_sb{h}", bufs=1)
                nc.vector.tensor_scalar(ce_sb[:], ctp[:, 0:DIM], rc[:], None, MUL)
                ce.append(ce_sb)

            # ---- pos/neg centers via onehot matmuls ----
            # onehot[p, b] = (cid[batch b] == h*128 + p); each column has
            # exactly one nonzero, so centers.T @ onehot is a gather (fp16
            # rounds the gathered center values only).
            for rep_sb, out in ((cpr_sb, PCT), (cnr_sb, NCT)):
                pcps = tppool.tile([DIM, nbpc], f32, name="pcps", tag="pcps",
                                   bufs=1)
                for h in range(2):
                    oh2 = ohpool.tile([128, nbpc], f16, name="oh2")
                    nc.vector.tensor_scalar(
                        oh2[:], rep_sb[:], pcol_sb[:, h : h + 1], None, EQ
                    )
                    for q in range(0, nbpc, 512):
                        qe = min(q + 512, nbpc)
                        nc.tensor.matmul(
                            pcps[:, q:qe], ce[h][:], oh2[:, q:qe],
                            start=(h == 0), stop=(h == 1),
                        )
                pct_sb = outpool.tile([DIM, nbpc], f32, name="pct_sb")
                nc.vector.tensor_copy(pct_sb[:], pcps[:])
                nc.scalar.dma_start(out[:], pct_sb[:])

            # ---- finalize uni_center ----
            p_sb = outpool.tile([MAUG, upc], f32, bufs=1)
            nc.vector.tensor_copy(p_sb[:, 0:nA], accA[:])
            if accB is not None:
                nc.vector.tensor_copy(p_sb[:, nA:upc], accB[:])
            for bi, (o, blk) in enumerate(blocks):
                ptp = tppool.tile([128, MAUG], f32, name="ptp", tag="tps")
                nc.tensor.matmul(
                    ptp[0:blk, :], p_sb[:, o : o + blk], eye_sb[:],
                    is_transpose=True,
                )
                r = outpool.tile([128, 1], f32, name="recip")
                nc.vector.reciprocal(r[0:blk, :], ptp[0:blk, 64:65])
                u_sb = outpool.tile([128, DIM], f32, name="u_sb")
                nc.vector.tensor_scalar(
                    u_sb[0:blk, :], ptp[0:blk, 0:DIM], r[0:blk, :], None, MUL
                )
                eng = nc.sync if bi % 2 == 0 else nc.scalar
                eng.dma_start(UNI[o : o + blk, :], u_sb[0:blk, :])

    split_multiwaits(nc)
    return nc


# ------------------------- host side -------------------------

def _wrap_idx(idx: np.ndarray) -> np.ndarray:
    """indirect gather layout: element [p, j] = idx[j*128 + p]."""
    n = idx.shape[0]
    return np.ascontiguousarray(idx.astype(np.int32).reshape(n // 128, 128).T)


def host_prep(user, pos, neg, cluster_ids, user_table, item_table, train_label):
    user = np.asarray(user).astype(np.int64)
    pos = np.asarray(pos).astype(np.int64)
    neg = np.asarray(neg).astype(np.int64)
    cluster_ids = np.asarray(cluster_ids).astype(np.int64)
    user_table = np.ascontiguousarray(np.asarray(user_table, dtype=np.float32))
    item_table = np.ascontiguousarray(np.asarray(item_table, dtype=np.float32))
    train_label = np.asarray(train_label, dtype=np.float32)

    uniq, inverse = np.unique(user, return_inverse=True)
    nu = len(uniq)
    upc = -(-nu // (NCORES * 16)) * 16  # per-core users, mult of 16
    upad = upc * NCORES
    uu = np.concatenate([uniq, np.full(upad - nu, uniq[0], dtype=uniq.dtype)])

    # T_aug partition-major [128, KCHUNKS, MAUG] bf16
    t_aug = np.zeros((KPAD, MAUG), np.float32)
    t_aug[: NUM_ITEMS, :DIM] = item_table[:NUM_ITEMS]  # row 20000 zeroed
    t_aug[: NUM_ITEMS + 1, DIM] = 1.0
    t_pm = np.ascontiguousarray(
        t_aug.reshape(KCHUNKS, 128, MAUG).transpose(1, 0, 2).reshape(128, -1)
    ).astype(ml_dtypes.bfloat16)
    # centers variant of the last chunk: real row 20000
    last = np.zeros((128, MAUG), np.float32)
    lo = (KCHUNKS - 1) * 128
    nreal = NUM_ITEMS + 1 - lo
    last[:nreal, :DIM] = item_table[lo : NUM_ITEMS + 1]
    last[:nreal, DIM] = 1.0
    t_cl = last.astype(ml_dtypes.bfloat16)

    eye66 = np.eye(MAUG, dtype=np.float32)
    iota256 = np.broadcast_to(
        np.arange(CLUSTER, dtype=np.float32), (128, CLUSTER)
    ).copy()
    cid_pm = np.full((KPAD,), -1.0, np.float32)
    cid_pm[: NUM_ITEMS + 1] = cluster_ids.astype(np.float32)
    cid_pm = np.ascontiguousarray(cid_pm.reshape(KCHUNKS, 128).T)
    pcol = (np.arange(128, dtype=np.float32)[:, None]
            + np.array([0.0, 128.0], np.float32)[None, :])
    pcol = np.ascontiguousarray(pcol)

    cpos = cluster_ids[pos].astype(np.float32)
    cneg = cluster_ids[neg].astype(np.float32)

    nbpc = BATCH // NCORES
    shared = {
        "t_pm": t_pm,
        "t_cl": t_cl,
        "eye66": eye66,
        "iota256": iota256,
        "cid_pm": cid_pm,
        "pcol": pcol,
        "user_table": user_table,
        "item_table": item_table,
    }
    in_maps = []
    for c in range(NCORES):
        rows = uu[c * upc : (c + 1) * upc]
        gathered = train_label[rows]  # [upc, 20001]
        lt = np.zeros((KPAD, upc), np.float32)
        lt[: NUM_ITEMS + 1, :] = gathered.T
        # partition-major: ltpm[p, c, u] = lt[c*128 + p, u]
        ltpm = np.ascontiguousarray(
            lt.reshape(KCHUNKS, 128, upc).transpose(1, 0, 2)
        )
        bs = slice(c * nbpc, (c + 1) * nbpc)
        m = dict(shared)
        m["lt"] = ltpm
        m["uidx"] = _wrap_idx(user[bs])
        m["pidx"] = _wrap_idx(pos[bs])
        m["nidx"] = _wrap_idx(neg[bs])
        m["cpr"] = np.ascontiguousarray(
            np.broadcast_to(cpos[bs][None, :], (128, nbpc))
        ).astype(ml_dtypes.bfloat16)
        m["cnr"] = np.ascontiguousarray(
            np.broadcast_to(cneg[bs][None, :], (128, nbpc))
        ).astype(ml_dtypes.bfloat16)
        in_maps.append(m)

    meta = {"upc": upc, "nbpc": nbpc, "nu": nu, "inverse": inverse}
    return in_maps, meta


def assemble(results, meta):
    inverse = meta["inverse"]
    uni_unique = np.concatenate([r["uni_part"] for r in results], axis=0)
    uni = uni_unique[inverse]
    ue = np.concatenate([r["ue_out"] for r in results], axis=0)
    pe = np.concatenate([r["pe_out"] for r in results], axis=0)
    ne = np.concatenate([r["ne_out"] for r in results], axis=0)
    pc = np.concatenate([r["pct_out"].T for r in results], axis=0)
    ncen = np.concatenate([r["nct_out"].T for r in results], axis=0)
    return ue, pe, ne, pc, ncen, uni


_CACHE = {}


def _run(in_maps, meta, trace=False):
    from concourse.bass_utils import run_bass_kernel_spmd

    key = (meta["upc"], meta["nbpc"])
    if key not in _CACHE:
        _CACHE[key] = build_bass(*key)
    nc = _CACHE[key]
    res = run_bass_kernel_spmd(
        nc, in_maps, core_ids=list(range(NCORES)), trace=trace
    )
    return res


def kernel(user, pos, neg, cluster_ids, user_table, item_table, train_label):
    """Full (unsharded) inputs -> full outputs, computed on 8 NeuronCores."""
    in_maps, meta = host_prep(
        user, pos, neg, cluster_ids, user_table, item_table, train_label
    )
    res = _run(in_maps, meta)
    return assemble(res.results, meta)



# revision 17
# speedup vs baseline: 1.7706x; 1.7706x over previous
"""Trainium2 Bass kernel for nn_MatrixFactorization (segment_reduce).

Decomposition (8 cores, SPMD, no collectives):
  - Dedup users of the batch -> unique users, sharded 8 ways (upc per core).
  - Host packs train_label[uniq].T per core as fp8e4m3 (labels are exactly
    0/1, so fp8 is lossless) in partition-major layout [128, 157, upc]:
    items land on SBUF partitions, contraction runs over 157 chunks of 128.
  - All big matmuls are FLIPPED vs the obvious orientation: the wide
    label/onehot operand is the STATIONARY side (lhsT) and the narrow
    [128, 65] item matrix (64 dims + ones column) is the MOVING side, so
    PE time scales with 65 columns per matmul, not with users/clusters.
      uni:     psum[128u, 65] += lt[:, c, b*128:...].T @ t_aug_c
      centers: psum[128c, 65] += onehot_c[:, h*128:...].T @ t_aug_c
    The ones column accumulates num_rel / cluster counts.
  - Centers finalize: max(count,1), reciprocal, scale -> cen_fin bf16
    [128c, 64] per half; pos/neg centers are onehot-stationary matmuls
    against cen_fin (exact gathers up to bf16 rounding of the centers).
  - user/pos/neg embeddings: GPSIMD indirect row gathers (256B rows),
    interleaved with the streaming loop.
  - All outputs use partition-major DRAM layouts (fully contiguous per
    partition => big DMA descriptors); the host unshuffles for free.
"""

import numpy as np
import ml_dtypes

import concourse.bass as bass
import concourse.mybir as mybir
import concourse.tile as tile

NUM_USERS = 10000
NUM_ITEMS = 20000
DIM = 64
CLUSTER = 256
BATCH = 8192
NCORES = 8

KCHUNKS = 157            # ceil(20001 / 128)
KPAD = KCHUNKS * 128     # 20096
M = 65                   # 64 dims + ones col
GROUP_SIZES = [4, 8] + [16] * 8 + [8, 6, 2, 1]  # staggered spin-up/down
assert sum(GROUP_SIZES) == KCHUNKS


def split_multiwaits(nc):
    """nix-walrus accepts at most ONE sync-wait per instruction; Tile attaches
    many. Hoist all but the last wait onto single-wait NoOps inserted just
    before the instruction, on the same engine."""
    n_split = 0
    for f in nc.m.functions:
        for bb in f.blocks:
            il = list(bb.instructions)
            new = []
            changed = False
            for ins in il:
                si = ins.sync_info
                if si is not None and si.on_wait is not None and len(si.on_wait) > 1:
                    waits = list(si.on_wait)
                    for k, w in enumerate(waits[:-1]):
                        nop = mybir.InstNoOp(
                            name=f"{ins.name}-wsplit{k}", ins=[], outs=[]
                        )
                        nop.engine = ins.engine
                        nop.sync_info = mybir.SyncInfo(on_wait=[w], on_update=[])
                        new.append(nop)
                    ins.sync_info = mybir.SyncInfo(
                        on_wait=waits[-1:], on_update=list(si.on_update or [])
                    )
                    changed = True
                    n_split += 1
                new.append(ins)
            if changed:
                bb.instructions = new
    return n_split


def build_bass(upc: int, nbpc: int):
    """upc: unique users per core; nbpc: batch entries per core."""
    f32 = mybir.dt.float32
    bf16 = mybir.dt.bfloat16
    fp8 = mybir.dt.float8e4
    i32 = mybir.dt.int32
    EQ = mybir.AluOpType.is_equal
    MUL = mybir.AluOpType.mult
    MAX = mybir.AluOpType.max

    assert nbpc % 128 == 0
    jg = nbpc // 128          # embedding gather slices per tensor
    nub = -(-upc // 128)      # user blocks
    nbb = nbpc // 128         # batch blocks (pos/neg centers)

    nc = bass.Bass(trn_type="TRN2")

    # ---- I/O ----
    # lt is partition-major fp8: lt[p, c, u] = label[item c*128+p, uniq user u]
    LT = nc.dram_tensor("lt", [128, KCHUNKS, upc], fp8, kind="ExternalInput")
    T_pm = nc.dram_tensor("t_pm", [128, KCHUNKS, M], bf16, kind="ExternalInput")
    T_cl = nc.dram_tensor("t_cl", [128, M], bf16, kind="ExternalInput")
    IOTA = nc.dram_tensor("iota256", [128, CLUSTER], bf16, kind="ExternalInput")
    CID = nc.dram_tensor("cid_pm", [128, KCHUNKS], f32, kind="ExternalInput")
    PCOL = nc.dram_tensor("pcol", [128, 2], f32, kind="ExternalInput")
    CPR = nc.dram_tensor("cpr", [128, nbpc], bf16, kind="ExternalInput")
    CNR = nc.dram_tensor("cnr", [128, nbpc], bf16, kind="ExternalInput")
    UT = nc.dram_tensor("user_table", [NUM_USERS, DIM], f32, kind="ExternalInput")
    IT = nc.dram_tensor("item_table", [NUM_ITEMS + 1, DIM], f32, kind="ExternalInput")
    IDX = {}
    for nm in ("uidx", "pidx", "nidx"):
        IDX[nm] = nc.dram_tensor(nm, [128, jg], i32, kind="ExternalInput")

    # partition-major outputs; host unshuffles
    UNI = nc.dram_tensor("uni_part", [128, nub, DIM], f32, kind="ExternalOutput")
    EMB = {}
    for nm in ("ue_out", "pe_out", "ne_out"):
        EMB[nm] = nc.dram_tensor(nm, [128, jg, DIM], f32, kind="ExternalOutput")
    PCT = nc.dram_tensor("pct_out", [128, nbb, DIM], f32, kind="ExternalOutput")
    NCT = nc.dram_tensor("nct_out", [128, nbb, DIM], f32, kind="ExternalOutput")

    gmax = max(GROUP_SIZES)

    with tile.TileContext(nc) as tc:
        with (
            tc.tile_pool(name="const", bufs=1) as cpool,
            tc.tile_pool(name="ltp", bufs=3) as ltpool,
            tc.tile_pool(name="ohp", bufs=3) as ohpool,
            tc.tile_pool(name="acc", bufs=1, space="PSUM") as accpool,
            tc.tile_pool(name="outp", bufs=3) as outpool,
        ):
            # ---- constants into SBUF ----
            # t_aug pieces on the sync ring (piecewise so early chunks arrive
            # first); every small constant on the scalar ring.
            t_sb = cpool.tile([128, KCHUNKS, M], bf16)
            tb = 0
            for piece in (GROUP_SIZES[0], GROUP_SIZES[1], 32, KCHUNKS):
                te = min(tb + piece, KCHUNKS)
                nc.sync.dma_start(t_sb[:, tb:te, :], T_pm[:, tb:te, :])
                tb = te
                if tb == KCHUNKS:
                    break
            iota_sb = cpool.tile([128, CLUSTER], bf16)
            nc.scalar.dma_start(iota_sb[:], IOTA[:])
            cid_sb = cpool.tile([128, KCHUNKS], f32)
            nc.scalar.dma_start(cid_sb[:], CID[:])
            tcl_sb = cpool.tile([128, M], bf16)
            nc.scalar.dma_start(tcl_sb[:], T_cl[:])
            pcol_sb = cpool.tile([128, 2], f32)
            nc.scalar.dma_start(pcol_sb[:], PCOL[:])
            idx_sb = {}
            g_sb = {}
            for nm, h in IDX.items():
                s = cpool.tile([128, jg], i32, name=f"idx_{nm}")
                nc.scalar.dma_start(s[:], h[:])
                idx_sb[nm] = s
                g_sb[nm] = cpool.tile([128, jg, DIM], f32, name=f"g_{nm}")
            cpr_sb = cpool.tile([128, nbpc], bf16)
            nc.scalar.dma_start(cpr_sb[:], CPR[:])
            cnr_sb = cpool.tile([128, nbpc], bf16)
            nc.scalar.dma_start(cnr_sb[:], CNR[:])

            # one [128]-row slice of an embedding gather
            gsrc = {"uidx": UT[:], "pidx": IT[:], "nidx": IT[:]}

            def gather_slice(nm, j):
                nc.gpsimd.indirect_dma_start(
                    out=g_sb[nm][:, j, :],
                    out_offset=None,
                    in_=gsrc[nm],
                    in_offset=bass.IndirectOffsetOnAxis(
                        ap=idx_sb[nm][:, j : j + 1], axis=0
                    ),
                )

            gather_slices = [(nm, j) for nm in ("uidx", "pidx", "nidx")
                             for j in range(jg)]

            # uni output staging; zeroed up front so the ragged last user
            # block can skip its pad rows
            uni_sb = outpool.tile([128, nub, DIM], f32, name="uni_sb", bufs=1)
            nc.vector.memset(uni_sb[:], 0.0)

            # ---- psum accumulators ----
            # a matmul start=True zeroes its whole 2KB PSUM bank, so every
            # accumulation group gets its own bank: 6 uni + 2 centers = all 8.
            # The pos/neg center matmuls later reuse the two center banks.
            uni_ps = [accpool.tile([128, M], f32, name=f"uni_ps{b}")
                      for b in range(nub)]
            cen_ps = [accpool.tile([128, M], f32, name=f"cen_ps{h}")
                      for h in range(2)]

            # pos/neg onehots [128c(half), nbpc] built once, mid-loop (below)
            ohb = {}

            # ---- main streaming loop ----
            c0 = 0
            n_groups = len(GROUP_SIZES)
            for g, gs in enumerate(GROUP_SIZES):
                lt = ltpool.tile([128, gmax, upc], fp8, name="lt_tile")
                nc.gpsimd.dma_start(lt[:, 0:gs, :], LT[:, c0 : c0 + gs, :])
                # spread the embedding-row gathers through the stream so
                # their descriptor generation hides under the big loads
                if g >= 1:
                    k0 = (g - 1) * len(gather_slices) // (n_groups - 1)
                    k1 = g * len(gather_slices) // (n_groups - 1)
                    for nm, j in gather_slices[k0:k1]:
                        gather_slice(nm, j)
                if g == 2:
                    # pos/neg center onehots: no loop deps; DVE has slack here
                    for key, rep_sb in (("p", cpr_sb), ("n", cnr_sb)):
                        for h in range(2):
                            t = cpool.tile([128, nbpc], bf16, name=f"ohb_{key}{h}")
                            nc.vector.tensor_scalar(
                                t[:], rep_sb[:], pcol_sb[:, h : h + 1], None, EQ
                            )
                            ohb[(key, h)] = t
                for j in range(gs):
                    c = c0 + j
                    st = c == 0
                    sp = c == KCHUNKS - 1
                    rhs = t_sb[:, c, :]
                    oh = ohpool.tile([128, CLUSTER], bf16, name="oh")
                    nc.vector.tensor_scalar(
                        oh[:], iota_sb[:], cid_sb[:, c : c + 1], None, EQ
                    )
                    crhs = tcl_sb[:] if sp else rhs
                    for h in range(2):
                        nc.tensor.matmul(
                            cen_ps[h][:], oh[:, h * 128 : (h + 1) * 128],
                            crhs, start=st, stop=sp,
                        )
                    for b in range(nub):
                        bw = min(128, upc - b * 128)
                        nc.tensor.matmul(
                            uni_ps[b][0:bw, :],
                            lt[:, j, b * 128 : b * 128 + bw],
                            rhs,
                            start=st, stop=sp,
                        )
                c0 += gs

            # ---- embedding gather writeback (flat partition-major) ----
            for nm, out in (("uidx", EMB["ue_out"]), ("pidx", EMB["pe_out"]),
                            ("nidx", EMB["ne_out"])):
                nc.scalar.dma_start(out[:], g_sb[nm][:])

            # ---- finalize centers -> cen_fin bf16 [128c, 64] per half ----
            cen_fin = []
            for h in range(2):
                cs = outpool.tile([128, M], f32, name=f"cs{h}", bufs=1)
                nc.vector.tensor_copy(cs[:], cen_ps[h][:])
                nc.vector.tensor_scalar(
                    cs[:, 64:65], cs[:, 64:65], 1.0, None, MAX
                )
                rc = outpool.tile([128, 1], f32, name=f"crc{h}")
                nc.vector.reciprocal(rc[:], cs[:, 64:65])
                cf = outpool.tile([128, DIM], bf16, name=f"cf{h}", bufs=1)
                nc.vector.tensor_scalar(cf[:], cs[:, 0:DIM], rc[:], None, MUL)
                cen_fin.append(cf)

            # ---- pos/neg centers: onehot-stationary gather matmuls ----
            # reuse the two freed center banks as rotating accumulators
            for ri, (key, out) in enumerate((("p", PCT), ("n", NCT))):
                pc_sb = outpool.tile([128, nbb, DIM], f32, name=f"pc_{key}",
                                     bufs=1)
                for b in range(nbb):
                    ps = cen_ps[b % 2]
                    for h in range(2):
                        nc.tensor.matmul(
                            ps[:, 0:DIM],
                            ohb[(key, h)][:, b * 128 : (b + 1) * 128],
                            cen_fin[h][:],
                            start=(h == 0), stop=(h == 1),
                        )
                    nc.vector.tensor_copy(pc_sb[:, b, :], ps[:, 0:DIM])
                eng = nc.sync if key == "p" else nc.scalar
                eng.dma_start(out[:], pc_sb[:])

            # ---- finalize uni_center ----
            for b in range(nub):
                bw = min(128, upc - b * 128)
                r = outpool.tile([128, 1], f32, name="urc")
                nc.vector.reciprocal(r[0:bw, :], uni_ps[b][0:bw, 64:65])
                nc.vector.tensor_scalar(
                    uni_sb[0:bw, b, :], uni_ps[b][0:bw, 0:DIM], r[0:bw, :],
                    None, MUL,
                )
            nc.sync.dma_start(UNI[:], uni_sb[:])

    split_multiwaits(nc)
    return nc


# ------------------------- host side -------------------------

def _wrap_idx(idx: np.ndarray) -> np.ndarray:
    """indirect gather layout: element [p, j] = idx[j*128 + p]."""
    n = idx.shape[0]
    return np.ascontiguousarray(idx.astype(np.int32).reshape(n // 128, 128).T)


def host_prep(user, pos, neg, cluster_ids, user_table, item_table, train_label):
    user = np.asarray(user).astype(np.int64)
    pos = np.asarray(pos).astype(np.int64)
    neg = np.asarray(neg).astype(np.int64)
    cluster_ids = np.asarray(cluster_ids).astype(np.int64)
    user_table = np.ascontiguousarray(np.asarray(user_table, dtype=np.float32))
    item_table = np.ascontiguousarray(np.asarray(item_table, dtype=np.float32))
    train_label = np.asarray(train_label, dtype=np.float32)

    uniq, inverse = np.unique(user, return_inverse=True)
    nu = len(uniq)
    upc = -(-nu // (NCORES * 16)) * 16  # per-core users, mult of 16
    upad = upc * NCORES
    uu = np.concatenate([uniq, np.full(upad - nu, uniq[0], dtype=uniq.dtype)])

    # t_aug partition-major [128, KCHUNKS, M] bf16; row 20000 zeroed for uni
    t_aug = np.zeros((KPAD, M), np.float32)
    t_aug[:NUM_ITEMS, :DIM] = item_table[:NUM_ITEMS]
    t_aug[: NUM_ITEMS + 1, DIM] = 1.0
    t_pm = np.ascontiguousarray(
        t_aug.reshape(KCHUNKS, 128, M).transpose(1, 0, 2)
    ).astype(ml_dtypes.bfloat16)
    # centers variant of the last chunk: real row 20000
    last = np.zeros((128, M), np.float32)
    lo = (KCHUNKS - 1) * 128
    nreal = NUM_ITEMS + 1 - lo
    last[:nreal, :DIM] = item_table[lo : NUM_ITEMS + 1]
    last[:nreal, DIM] = 1.0
    t_cl = last.astype(ml_dtypes.bfloat16)

    iota256 = np.broadcast_to(
        np.arange(CLUSTER, dtype=np.float32), (128, CLUSTER)
    ).astype(ml_dtypes.bfloat16)
    cid_pm = np.full((KPAD,), -1.0, np.float32)
    cid_pm[: NUM_ITEMS + 1] = cluster_ids.astype(np.float32)
    cid_pm = np.ascontiguousarray(cid_pm.reshape(KCHUNKS, 128).T)
    pcol = (np.arange(128, dtype=np.float32)[:, None]
            + np.array([0.0, 128.0], np.float32)[None, :])
    pcol = np.ascontiguousarray(pcol)

    cpos = cluster_ids[pos].astype(np.float32)
    cneg = cluster_ids[neg].astype(np.float32)

    nbpc = BATCH // NCORES
    shared = {
        "t_pm": t_pm,
        "t_cl": t_cl,
        "iota256": iota256,
        "cid_pm": cid_pm,
        "pcol": pcol,
        "user_table": user_table,
        "item_table": item_table,
    }
    in_maps = []
    for c in range(NCORES):
        rows = uu[c * upc : (c + 1) * upc]
        gathered = train_label[rows]  # [upc, 20001] f32
        lt = np.zeros((KPAD, upc), ml_dtypes.float8_e4m3)
        lt[: NUM_ITEMS + 1, :] = gathered.T.astype(ml_dtypes.float8_e4m3)
        # partition-major: ltpm[p, c, u] = lt[c*128 + p, u]
        ltpm = np.ascontiguousarray(
            lt.reshape(KCHUNKS, 128, upc).transpose(1, 0, 2)
        )
        bs = slice(c * nbpc, (c + 1) * nbpc)
        m = dict(shared)
        m["lt"] = ltpm
        m["uidx"] = _wrap_idx(user[bs])
        m["pidx"] = _wrap_idx(pos[bs])
        m["nidx"] = _wrap_idx(neg[bs])
        m["cpr"] = np.ascontiguousarray(
            np.broadcast_to(cpos[bs][None, :], (128, nbpc))
        ).astype(ml_dtypes.bfloat16)
        m["cnr"] = np.ascontiguousarray(
            np.broadcast_to(cneg[bs][None, :], (128, nbpc))
        ).astype(ml_dtypes.bfloat16)
        in_maps.append(m)

    meta = {"upc": upc, "nbpc": nbpc, "nu": nu, "inverse": inverse}
    return in_maps, meta


def _unshuffle_pm(arr):
    """[128, nblk, 64] partition-major -> [nblk*128, 64] row-major."""
    return np.ascontiguousarray(arr.transpose(1, 0, 2)).reshape(-1, arr.shape[2])


def assemble(results, meta):
    inverse = meta["inverse"]
    upc = meta["upc"]
    uni_unique = np.concatenate(
        [_unshuffle_pm(r["uni_part"])[:upc] for r in results], axis=0
    )
    uni = uni_unique[inverse]
    ue = np.concatenate([_unshuffle_pm(r["ue_out"]) for r in results], axis=0)
    pe = np.concatenate([_unshuffle_pm(r["pe_out"]) for r in results], axis=0)
    ne = np.concatenate([_unshuffle_pm(r["ne_out"]) for r in results], axis=0)
    pc = np.concatenate([_unshuffle_pm(r["pct_out"]) for r in results], axis=0)
    ncen = np.concatenate([_unshuffle_pm(r["nct_out"]) for r in results], axis=0)
    return ue, pe, ne, pc, ncen, uni


_CACHE = {}


def _run(in_maps, meta, trace=False):
    from concourse.bass_utils import run_bass_kernel_spmd

    key = (meta["upc"], meta["nbpc"])
    if key not in _CACHE:
        _CACHE[key] = build_bass(*key)
    nc = _CACHE[key]
    res = run_bass_kernel_spmd(
        nc, in_maps, core_ids=list(range(NCORES)), trace=trace
    )
    return res


def kernel(user, pos, neg, cluster_ids, user_table, item_table, train_label):
    """Full (unsharded) inputs -> full outputs, computed on 8 NeuronCores."""
    in_maps, meta = host_prep(
        user, pos, neg, cluster_ids, user_table, item_table, train_label
    )
    res = _run(in_maps, meta)
    return assemble(res.results, meta)


# revision 68
# speedup vs baseline: 2.8423x; 1.6053x over previous
"""Trainium2 Bass kernel for nn_MatrixFactorization (segment_reduce).

Decomposition (8 cores, SPMD, no collectives):
  - Dedup users of the batch -> unique users, sharded 8 ways (upc per core).
  - Host sorts items by cluster id and packs train_label[uniq].T (item axis
    permuted) as fp8e4m3 (labels are exactly 0/1, so fp8 is lossless) in
    partition-major layout [128, 157, upc]; the label stream is split
    round-robin across the THREE DMA rings (SWDGE/Pool, SP, Activation),
    which pipeline independently, into one fully-resident SBUF buffer.
  - The item matrix rides as fp8 hi+lo pairs t2[128, c, 130] = [fp8(T) |
    fp8(T - fp8(T))] (64 dims + ones column per half); hi+lo recovers
    ~bf16 accuracy while keeping 2 bytes/value.
  - All big matmuls are FLIPPED vs the obvious orientation: the wide
    label/onehot operand is the STATIONARY side (lhsT) and the narrow
    item matrix the MOVING side, so PE time scales with 130 columns per
    matmul, not with users/clusters:
      uni:     DoubleRow fp8 pairs: psum[128u, 130] +=
                 sum_i lt[:, 2j+i, b*128:...].T @ t2[:, 2j+i, :]
               (two K-chunks per matmul at 0.5 cycles/row)
      centers: psum[128c, 130] += onehot_c.T @ t2[:, c, :]
    The ones column accumulates num_rel / cluster counts; hi+lo halves
    are summed on DVE at finalize.
  - Items sorted by cluster => each chunk touches one 128-cluster half
    (two only for the single straddle chunk): half-width onehots, one
    center matmul per chunk, and the half-0 center bank closes mid-stream
    so its finalize + pos-center h0 matmuls hide under the stream.
  - user/pos/neg embeddings: batched GPSIMD indirect row gathers issued
    after the label stream so their DMA hides under the tail chain.
  - All outputs are bf16 in partition-major DRAM layouts; host unshuffles
    and upcasts.
"""

import numpy as np
import ml_dtypes

import concourse.bass as bass
import concourse.mybir as mybir
import concourse.tile as tile

NUM_USERS = 10000
NUM_ITEMS = 20000
DIM = 64
CLUSTER = 256
BATCH = 8192
NCORES = 8

KCHUNKS = 157            # ceil(20001 / 128)
KPAD = KCHUNKS * 128     # 20096
M = 65                   # 64 dims + ones col
M2 = 2 * M               # hi | lo
NPAIR = KCHUNKS // 2     # 78 DoubleRow pairs + 1 single chunk
# even-sized groups so DoubleRow pairs never wait on two DMAs
GROUP_SIZES = [4, 8, 20, 24, 16, 20, 24, 12, 28, 1]
assert sum(GROUP_SIZES) == KCHUNKS
# ring index per group: 0=Pool(SWDGE) 1=SP 2=Activation. Pool also carries
# t2 + cpn + the gathers, SP carries t1, Act the packed constants; the
# heavier lt share goes to Act.
GROUP_RING = [0, 2, 1, 2, 1, 0, 1, 2, 2, 0]
assert len(GROUP_RING) == len(GROUP_SIZES)


def split_multiwaits(nc):
    """nix-walrus accepts at most ONE sync-wait per instruction; Tile attaches
    many. Hoist all but the last wait onto single-wait NoOps inserted just
    before the instruction, on the same engine."""
    n_split = 0
    for f in nc.m.functions:
        for bb in f.blocks:
            il = list(bb.instructions)
            new = []
            changed = False
            for ins in il:
                si = ins.sync_info
                if si is not None and si.on_wait is not None and len(si.on_wait) > 1:
                    waits = list(si.on_wait)
                    for k, w in enumerate(waits[:-1]):
                        nop = mybir.InstNoOp(
                            name=f"{ins.name}-wsplit{k}", ins=[], outs=[]
                        )
                        nop.engine = ins.engine
                        nop.sync_info = mybir.SyncInfo(on_wait=[w], on_update=[])
                        new.append(nop)
                    ins.sync_info = mybir.SyncInfo(
                        on_wait=waits[-1:], on_update=list(si.on_update or [])
                    )
                    changed = True
                    n_split += 1
                new.append(ins)
            if changed:
                bb.instructions = new
    return n_split


def build_bass(upc: int, nbpc: int, hlist: tuple, straddle: tuple, cq: int):
    """upc: unique users per core; nbpc: batch entries per core.
    hlist[c]: 128-cluster half touched by (sorted) chunk c; straddle[c]: chunk
    also touches half hlist[c]+1; cq: chunk holding item 20000 (centers use
    the t_cl variant there)."""
    f32 = mybir.dt.float32
    bf16 = mybir.dt.bfloat16
    fp8 = mybir.dt.float8e4
    i32 = mybir.dt.int32
    EQ = mybir.AluOpType.is_equal
    MUL = mybir.AluOpType.mult
    MAX = mybir.AluOpType.max
    ADD = mybir.AluOpType.add
    DR = mybir.MatmulPerfMode.DoubleRow

    assert nbpc % 128 == 0
    jg = nbpc // 128          # embedding gather slices per tensor
    nub = -(-upc // 128)      # user blocks
    nbb = nbpc // 128         # batch blocks (pos/neg centers)

    # chunk -> center-matmul halves
    contrib = [[] for _ in range(KCHUNKS)]
    for c in range(KCHUNKS):
        contrib[c].append(hlist[c])
        if straddle[c]:
            contrib[c].append(hlist[c] + 1)
    chunks_of = {h: [c for c in range(KCHUNKS) if h in contrib[c]]
                 for h in range(2)}
    assert chunks_of[0] and chunks_of[1]
    stop0 = chunks_of[0][-1]

    nc = bass.Bass(trn_type="TRN2")

    # ---- I/O ----
    # lt is partition-major fp8 over SORTED items:
    #   lt[p, c, u] = label[perm[c*128+p], uniq user u]
    LT = nc.dram_tensor("lt", [128, KCHUNKS, upc], fp8, kind="ExternalInput")
    T2 = nc.dram_tensor("t2", [128, KCHUNKS, M2], fp8, kind="ExternalInput")
    T1 = nc.dram_tensor("t1", [128, KCHUNKS, M], bf16, kind="ExternalInput")
    IOTA = nc.dram_tensor("iota256", [128, CLUSTER], bf16, kind="ExternalInput")
    CIDP = nc.dram_tensor("cidp", [128, KCHUNKS + 2], f32, kind="ExternalInput")
    CPN = nc.dram_tensor("cpn", [128, 2 * nbpc], bf16, kind="ExternalInput")
    UT = nc.dram_tensor("ut_bf", [NUM_USERS, DIM], bf16, kind="ExternalInput")
    IT = nc.dram_tensor("it_bf", [NUM_ITEMS + 1, DIM], bf16, kind="ExternalInput")
    IDX = {}
    for nm in ("uidx", "pidx", "nidx"):
        IDX[nm] = nc.dram_tensor(nm, [128, jg], i32, kind="ExternalInput")

    # partition-major bf16 outputs; host unshuffles and upcasts
    UNI = nc.dram_tensor("uni_part", [128, nub, DIM], bf16, kind="ExternalOutput")
    EMB = {}
    for nm in ("ue_out", "pe_out", "ne_out"):
        EMB[nm] = nc.dram_tensor(nm, [128, jg, DIM], bf16, kind="ExternalOutput")
    PCT = nc.dram_tensor("pct_out", [128, nbb, DIM], bf16, kind="ExternalOutput")
    NCT = nc.dram_tensor("nct_out", [128, nbb, DIM], bf16, kind="ExternalOutput")

    with tile.TileContext(nc) as tc:
        with (
            tc.tile_pool(name="const", bufs=1) as cpool,
            tc.tile_pool(name="ohp", bufs=4) as ohpool,
            tc.tile_pool(name="acc", bufs=1, space="PSUM") as accpool,
            tc.tile_pool(name="outp", bufs=3) as outpool,
        ):
            # ---- packed constants FIRST on the Act ring ----
            cidp_sb = cpool.tile([128, KCHUNKS + 2], f32, name="cidp")
            nc.scalar.dma_start(cidp_sb[:], CIDP[:])
            iota_sb = cpool.tile([128, CLUSTER], bf16)
            nc.scalar.dma_start(iota_sb[:], IOTA[:])
            cid_sb = cidp_sb  # cid = [:, 0:KCHUNKS], pcol = [:, KCHUNKS:+2]
            idx_sb = {}
            g_sb = {}
            for nm, h in IDX.items():
                t = cpool.tile([128, jg], i32, name=f"idx_{nm}")
                nc.scalar.dma_start(t[:], h[:])
                idx_sb[nm] = t
                g_sb[nm] = cpool.tile([128, jg, DIM], bf16, name=f"g_{nm}")

            # ---- fully-resident label buffer + t1/t2, streamed on 3 rings:
            # t2 pieces early on Pool (gates uni), t1 on SP (gates centers),
            # labels round-robin with Act carrying the largest share ----
            lt_sb = cpool.tile([128, KCHUNKS, upc], fp8, name="lt_sb")
            t1_sb = cpool.tile([128, KCHUNKS, M], bf16, name="t1_sb")
            t2_sb = cpool.tile([128, KCHUNKS, M2], fp8, name="t2_sb")
            cpn_sb = cpool.tile([128, 2 * nbpc], bf16, name="cpn_sb")

            rings = [nc.gpsimd, nc.sync, nc.scalar]
            c0s = np.cumsum([0] + GROUP_SIZES[:-1]).tolist()
            nc.gpsimd.dma_start(t2_sb[:, 0:24, :], T2[:, 0:24, :])
            nc.sync.dma_start(t1_sb[:, 0:24, :], T1[:, 0:24, :])
            nc.sync.dma_start(t1_sb[:, 24:96, :], T1[:, 24:96, :])
            for g, gs in enumerate(GROUP_SIZES):
                c0 = c0s[g]
                rings[GROUP_RING[g]].dma_start(
                    lt_sb[:, c0 : c0 + gs, :], LT[:, c0 : c0 + gs, :]
                )
                if g == 0:
                    nc.gpsimd.dma_start(t2_sb[:, 24:72, :], T2[:, 24:72, :])
                    nc.gpsimd.dma_start(t2_sb[:, 72:KCHUNKS, :],
                                        T2[:, 72:KCHUNKS, :])
                    nc.sync.dma_start(t1_sb[:, 96:KCHUNKS, :],
                                      T1[:, 96:KCHUNKS, :])
                if g == 5:
                    nc.gpsimd.dma_start(cpn_sb[:], CPN[:])

            # uni output staging; zeroed up front so the ragged last user
            # block can skip its pad rows
            uni_sb = outpool.tile([128, nub, DIM], bf16, name="uni_sb", bufs=1)
            nc.vector.memset(uni_sb[:], 0.0)

            # ---- psum accumulators: 6 uni + 2 centers = all 8 banks ----
            uni_ps = [accpool.tile([128, 512], f32, name=f"uni_ps{b}")
                      for b in range(nub)]
            cen_ps = [accpool.tile([128, 512], f32, name=f"cen_ps{h}")
                      for h in range(2)]

            # embedding gathers: [128, 1]-offset slices (the only indirect
            # form walrus codegen handles correctly); spread through the
            # pair loop so the SWDGE generation hides under the stream
            gsrc = {"uidx": UT[:], "pidx": IT[:], "nidx": IT[:]}

            def gather_slice(nm, j):
                nc.gpsimd.indirect_dma_start(
                    out=g_sb[nm][:, j, :],
                    out_offset=None,
                    in_=gsrc[nm],
                    in_offset=bass.IndirectOffsetOnAxis(
                        ap=idx_sb[nm][:, j : j + 1], axis=0
                    ),
                )

            gather_slices = [(nm, j) for nm in ("uidx", "pidx", "nidx")
                             for j in range(jg)]

            ohb = {}
            cen_fin = [None, None]
            cen_started = [False, False]
            pc_specs = (("p", 0, PCT), ("n", 1, NCT))

            def emit_cen(c):
                # one (or two, straddle) half-width onehot center matmuls
                # against the narrow bf16 item matrix (which carries the
                # real item-20000 row, unlike t2's uni variant)
                rhs = t1_sb[:, c, :]
                for h in contrib[c]:
                    oh = ohpool.tile([128, 128], bf16, name="oh")
                    nc.vector.tensor_scalar(
                        oh[:], iota_sb[:, h * 128 : (h + 1) * 128],
                        cid_sb[:, c : c + 1], None, EQ,
                    )
                    nc.tensor.matmul(
                        cen_ps[h][:, 0:M], oh[:], rhs,
                        start=not cen_started[h],
                        stop=(c == chunks_of[h][-1]),
                    )
                    cen_started[h] = True

            def finalize_centers(h):
                # max(count,1), reciprocal, scale -> bf16
                cs = outpool.tile([128, M], f32, name=f"cs{h}", bufs=1)
                nc.vector.tensor_copy(cs[:], cen_ps[h][:, 0:M])
                nc.vector.tensor_scalar(
                    cs[:, 64:65], cs[:, 64:65], 1.0, None, MAX
                )
                rc = outpool.tile([128, 1], f32, name=f"crc{h}")
                nc.vector.reciprocal(rc[:], cs[:, 64:65])
                cf = outpool.tile([128, DIM], bf16, name=f"cf{h}", bufs=1)
                nc.vector.tensor_scalar(cf[:], cs[:, 0:DIM], rc[:], None, MUL)
                cen_fin[h] = cf

            def pc_matmuls(key, ps, hs, start):
                for hi, h in enumerate(hs):
                    for b in range(nbb):
                        nc.tensor.matmul(
                            ps[:, b * DIM : (b + 1) * DIM],
                            ohb[(key, h)][:, b * 128 : (b + 1) * 128],
                            cen_fin[h][:],
                            start=(start and hi == 0 and b == 0),
                            stop=(h == 1 and b == nbb - 1),
                        )

            # ---- main compute loop over DoubleRow pairs ----
            for j in range(NPAIR):
                emit_cen(2 * j)
                emit_cen(2 * j + 1)
                for b in range(nub):
                    bw = min(128, upc - b * 128)
                    nc.tensor.matmul(
                        uni_ps[b][0:bw, 0:M2],
                        lt_sb[:, 2 * j : 2 * j + 2, b * 128 : b * 128 + bw],
                        t2_sb[:, 2 * j : 2 * j + 2, :],
                        start=(j == 0), stop=False,
                        perf_mode=DR,
                    )
                if 6 <= j < 6 + len(gather_slices):
                    gather_slice(*gather_slices[j - 6])
                if j == 8:
                    # pos/neg center onehots (batch cluster ids vs partition
                    # index); DVE has slack mid-loop
                    for ki, key in enumerate(("p", "n")):
                        for h in range(2):
                            t = cpool.tile([128, nbpc], bf16,
                                           name=f"ohb_{key}{h}")
                            nc.vector.tensor_scalar(
                                t[:],
                                cpn_sb[:, ki * nbpc : (ki + 1) * nbpc],
                                cidp_sb[:, KCHUNKS + h : KCHUNKS + h + 1],
                                None, EQ,
                            )
                            ohb[(key, h)] = t
                if cen_fin[0] is None and 2 * j + 1 >= stop0 and j >= 10:
                    finalize_centers(0)
                    pc_matmuls("p", cen_ps[0], (0,), start=True)
            # final unpaired chunk
            c = KCHUNKS - 1
            emit_cen(c)
            for b in range(nub):
                bw = min(128, upc - b * 128)
                nc.tensor.matmul(
                    uni_ps[b][0:bw, 0:M2],
                    lt_sb[:, c, b * 128 : b * 128 + bw],
                    t2_sb[:, c, :],
                    start=False, stop=True,
                )

            # ---- tail: half-1 centers, uni finalize, pc close ----
            if cen_fin[0] is None:  # fallback: half 0 closed very late
                finalize_centers(0)
                pc_matmuls("p", cen_ps[0], (0,), start=True)
            finalize_centers(1)

            # uni finalize on DVE (overlaps the pc matmuls on PE):
            # hi+lo merge, reciprocal of num_rel, scale
            u2 = outpool.tile([128, M], f32, name="u2")
            r_all = outpool.tile([128, nub], f32, name="urc_all", bufs=1)
            for b in range(nub):
                bw = min(128, upc - b * 128)
                # DVE may read only ONE non-scalar PSUM input per op
                nc.vector.tensor_copy(u2[0:bw, :], uni_ps[b][0:bw, 0:M])
                nc.vector.tensor_tensor(
                    u2[0:bw, :], u2[0:bw, :], uni_ps[b][0:bw, M:M2], ADD
                )
                nc.vector.reciprocal(r_all[0:bw, b : b + 1], u2[0:bw, 64:65])
                nc.vector.tensor_scalar(
                    uni_sb[0:bw, b, :], u2[0:bw, 0:DIM],
                    r_all[0:bw, b : b + 1], None, MUL,
                )
            nc.sync.dma_start(UNI[:], uni_sb[:])

            pc_matmuls("p", cen_ps[0], (1,), start=False)
            # the neg rep waits for the cen1 bank: its h0+h1 matmuls both
            # run here, after the half-1 finalize frees the bank
            pc_matmuls("n", cen_ps[1], (0, 1), start=True)

            # embedding writebacks (gathers completed mid-loop)
            for eng, nm, out in ((nc.gpsimd, "uidx", EMB["ue_out"]),
                                 (nc.sync, "pidx", EMB["pe_out"]),
                                 (nc.scalar, "nidx", EMB["ne_out"])):
                eng.dma_start(out[:], g_sb[nm][:])

            # pc copies: pos on Activation, neg on DVE (parallel tails)
            for key, ri, out in pc_specs:
                pc_sb = outpool.tile([128, nbb * DIM], bf16, name=f"pc_{key}",
                                     bufs=1)
                if key == "p":
                    nc.scalar.copy(pc_sb[:], cen_ps[ri][:, 0 : nbb * DIM])
                else:
                    nc.vector.tensor_copy(pc_sb[:], cen_ps[ri][:, 0 : nbb * DIM])
                eng = nc.sync if key == "p" else nc.scalar
                eng.dma_start(out[:], pc_sb[:])

    split_multiwaits(nc)
    return nc


# ------------------------- host side -------------------------

def _wrap_idx(idx: np.ndarray) -> np.ndarray:
    """indirect gather layout: element [p, j] = idx[j*128 + p]."""
    n = idx.shape[0]
    return np.ascontiguousarray(idx.astype(np.int32).reshape(n // 128, 128).T)


def _hi_lo(t: np.ndarray):
    """fp8e4m3 hi + residual-lo decomposition (hi+lo ~ bf16 accuracy)."""
    hi = t.astype(ml_dtypes.float8_e4m3)
    lo = (t - hi.astype(np.float32)).astype(ml_dtypes.float8_e4m3)
    return hi, lo


def host_prep(user, pos, neg, cluster_ids, user_table, item_table, train_label):
    user = np.asarray(user).astype(np.int64)
    pos = np.asarray(pos).astype(np.int64)
    neg = np.asarray(neg).astype(np.int64)
    cluster_ids = np.asarray(cluster_ids).astype(np.int64)
    user_table = np.ascontiguousarray(np.asarray(user_table, dtype=np.float32))
    item_table = np.ascontiguousarray(np.asarray(item_table, dtype=np.float32))
    train_label = np.asarray(train_label, dtype=np.float32)

    uniq, inverse = np.unique(user, return_inverse=True)
    nu = len(uniq)
    # dual-fp8 Ldweights requires the k-row stride to be a multiple of 64
    # bytes, so pad the per-core user count to a multiple of 64
    upc = -(-nu // (NCORES * 64)) * 64
    upad = upc * NCORES
    uu = np.concatenate([uniq, np.full(upad - nu, uniq[0], dtype=uniq.dtype)])

    # sort items by cluster id (stable) so each chunk touches one half
    perm = np.argsort(cluster_ids, kind="stable").astype(np.int64)
    cs_cid = cluster_ids[perm]                 # ascending
    q = int(np.nonzero(perm == NUM_ITEMS)[0][0])
    cq = q // 128

    hlist, straddle = [], []
    for c in range(KCHUNKS):
        lo = c * 128
        hi = min(lo + 127, NUM_ITEMS)
        h0 = int(cs_cid[lo]) // 128
        h1 = int(cs_cid[hi]) // 128
        hlist.append(h0)
        straddle.append(h1 != h0)
    hlist, straddle = tuple(hlist), tuple(straddle)

    # t_aug over sorted items: fp8 hi|lo halves t2 [128, KCHUNKS, 130] for
    # the uni DoubleRow matmuls, narrow bf16 t1 [128, KCHUNKS, 65] for the
    # center matmuls; item 20000's dims zeroed for uni
    t_aug = np.zeros((KPAD, M), np.float32)
    t_aug[: NUM_ITEMS + 1, :DIM] = item_table[perm]
    t_aug[q, :DIM] = 0.0
    t_aug[: NUM_ITEMS + 1, DIM] = 1.0
    hi, lo = _hi_lo(t_aug)
    t2 = np.concatenate([hi.reshape(KCHUNKS, 128, M),
                         lo.reshape(KCHUNKS, 128, M)], axis=2)
    t2 = np.ascontiguousarray(t2.transpose(1, 0, 2))  # [128, KCHUNKS, 130]

    # centers variant: real item 20000 dims (t1 + the cq-chunk override)
    t_cen = t_aug.copy()
    t_cen[q, :DIM] = item_table[NUM_ITEMS]
    t1 = np.ascontiguousarray(
        t_cen.reshape(KCHUNKS, 128, M).transpose(1, 0, 2)
    ).astype(ml_dtypes.bfloat16)

    iota256 = np.broadcast_to(
        np.arange(CLUSTER, dtype=np.float32), (128, CLUSTER)
    ).astype(ml_dtypes.bfloat16)
    cid_pm = np.full((KPAD,), -1.0, np.float32)
    cid_pm[: NUM_ITEMS + 1] = cs_cid.astype(np.float32)
    cid_pm = cid_pm.reshape(KCHUNKS, 128).T
    pcol = (np.arange(128, dtype=np.float32)[:, None]
            + np.array([0.0, 128.0], np.float32)[None, :])
    cidp = np.ascontiguousarray(np.concatenate([cid_pm, pcol], axis=1))

    cpos = cluster_ids[pos].astype(np.float32)
    cneg = cluster_ids[neg].astype(np.float32)

    nbpc = BATCH // NCORES
    shared = {
        "t1": t1,
        "t2": t2,
        "iota256": iota256,
        "cidp": cidp,
        "ut_bf": user_table.astype(ml_dtypes.bfloat16),
        "it_bf": item_table.astype(ml_dtypes.bfloat16),
    }
    in_maps = []
    for c in range(NCORES):
        rows = uu[c * upc : (c + 1) * upc]
        gathered = train_label[rows]  # [upc, 20001] f32
        lt = np.zeros((KPAD, upc), ml_dtypes.float8_e4m3)
        lt[: NUM_ITEMS + 1, :] = gathered.T[perm].astype(ml_dtypes.float8_e4m3)
        ltpm = np.ascontiguousarray(
            lt.reshape(KCHUNKS, 128, upc).transpose(1, 0, 2)
        )
        bs = slice(c * nbpc, (c + 1) * nbpc)
        m = dict(shared)
        m["lt"] = ltpm
        m["uidx"] = _wrap_idx(user[bs])
        m["pidx"] = _wrap_idx(pos[bs])
        m["nidx"] = _wrap_idx(neg[bs])
        m["cpn"] = np.ascontiguousarray(np.broadcast_to(
            np.concatenate([cpos[bs], cneg[bs]])[None, :], (128, 2 * nbpc)
        )).astype(ml_dtypes.bfloat16)
        in_maps.append(m)

    meta = {"upc": upc, "nbpc": nbpc, "nu": nu, "inverse": inverse,
            "hlist": hlist, "straddle": straddle, "cq": cq}
    return in_maps, meta


def _unshuffle_pm(arr):
    """[128, nblk, 64] partition-major -> [nblk*128, 64] row-major f32."""
    arr = np.asarray(arr, dtype=np.float32)
    return np.ascontiguousarray(arr.transpose(1, 0, 2)).reshape(-1, arr.shape[2])


def assemble(results, meta):
    inverse = meta["inverse"]
    upc = meta["upc"]
    uni_unique = np.concatenate(
        [_unshuffle_pm(r["uni_part"])[:upc] for r in results], axis=0
    )
    uni = uni_unique[inverse]
    ue = np.concatenate([_unshuffle_pm(r["ue_out"]) for r in results], axis=0)
    pe = np.concatenate([_unshuffle_pm(r["pe_out"]) for r in results], axis=0)
    ne = np.concatenate([_unshuffle_pm(r["ne_out"]) for r in results], axis=0)
    pc = np.concatenate(
        [_unshuffle_pm(r["pct_out"].reshape(128, -1, DIM)) for r in results],
        axis=0,
    )
    ncen = np.concatenate(
        [_unshuffle_pm(r["nct_out"].reshape(128, -1, DIM)) for r in results],
        axis=0,
    )
    return ue, pe, ne, pc, ncen, uni


_CACHE = {}


def build_from_meta(meta):
    return build_bass(meta["upc"], meta["nbpc"], meta["hlist"],
                      meta["straddle"], meta["cq"])


def _run(in_maps, meta, trace=False):
    from concourse.bass_utils import run_bass_kernel_spmd

    key = (meta["upc"], meta["nbpc"], meta["hlist"], meta["straddle"],
           meta["cq"])
    if key not in _CACHE:
        _CACHE[key] = build_from_meta(meta)
    nc = _CACHE[key]
    res = run_bass_kernel_spmd(
        nc, in_maps, core_ids=list(range(NCORES)), trace=trace
    )
    return res


def kernel(user, pos, neg, cluster_ids, user_table, item_table, train_label):
    """Full (unsharded) inputs -> full outputs, computed on 8 NeuronCores."""
    in_maps, meta = host_prep(
        user, pos, neg, cluster_ids, user_table, item_table, train_label
    )
    res = _run(in_maps, meta)
    return assemble(res.results, meta)
